# revision 34
# baseline (speedup 1.0000x reference)
"""BiLSTM-CRF Trainium2 kernel — 8-core TIME-chunked scan.

Contract: kernel(**inputs) takes the FULL unsharded inputs (numpy arrays,
keyed as in reference.setup_inputs()) and returns the FULL [B, T, TAGS, TAGS]
crf_scores array.

Sharding: the 512-step scan is latency-bound (per-step serial chain of
~2.2us across PE->ACT->VEC->VEC->ACT->VEC), and per-instruction costs are
almost entirely fixed overhead, so batching all 64 sequences into one op
costs barely more than 8.  We therefore split TIME, not batch: core c
computes time chunk [64c, 64c+64) for the full batch, running its fwd/bwd
scans W extra warmup steps from zero state.  The LSTM here is strongly
contractive (forget gate ~0.5), so the warmup truncation error decays
~0.6^W.  All 16-bit tensors are fp16 (not bf16): same PE throughput,
~8x less quantization noise, which pays for W=12 (vs 16) warmup:
measured rel-err 5.2e-3 (fp16,W=12) vs 1.49e-2 (bf16,W=16).

Cell math is reformulated tanh-only (sigma(x) = (tanh(x/2)+1)/2, with
weights pre-scaled on host, hidden state carried as h' = 2h and cell
state as cc = 2c):
    T = tanh(z')            z' blocks [o, i, f, g] with f,i,o halved
    P = [(T_i+1)*T_g | (T_f+1)*cc]      (one scalar_tensor_tensor)
    cc_new = 0.5*P_f + P_i              (one scalar_tensor_tensor,
                                         written into next step's Q tile)
    TC = tanh(0.5*cc_new)               (ACT free scale)
    h' = (T_o+1)*TC                     (one scalar_tensor_tensor)

Embedding gather: one SWDGE queue only (TRN2), ~725ns + 7.9ns/token per
gather, so the gathers are ordered strictly by the scan's first-use time
(fwd consumes blocks ascending, bwd descending, simultaneously), with
fine pieces (128-token quarters) at the start so the scan can begin
~19us in, coarsening to 256/512-token pieces once the scan is underway.
"""
import sys
import types
from contextlib import ExitStack

import numpy as np

import concourse.bacc as bacc
import concourse.bass as bass
import concourse.mybir as mybir
import concourse.tile as tile
from concourse import library_config
from concourse.bass_utils import run_bass_kernel_spmd

# ---- problem dims (hardcoded per spec) ----
VOCAB = 30000
VPAD = 30720      # emb table padded; row 30000 is all-zeros (warmup token)
ZTOK = 30000
EMB = 256
HD = 128          # per-direction hidden
G4 = 512          # 4*HD gates
TAGS = 16
B, T = 64, 512
NCORES = 8

CH = 64           # time chunk per core
W = 10            # warmup steps
S = CH + W        # scan steps per direction (74)
NTS = CH + 2 * W  # timesteps of tokens/zin per core (84)
NTOK = NTS * B    # tokens per core
TPB = 8           # timesteps per 512-token block
NB = NTS // TPB   # full gather/gemm blocks (10); ti 80-83 is a half-block

F16 = mybir.dt.float16
F32 = mybir.dt.float32
I16 = mybir.dt.int16
AF = mybir.ActivationFunctionType
ALU = mybir.AluOpType

# gate order in reference (jnp.split): i, f, g, o.  Device block order is
# [o, i, f, g] so that W-pair = [T_i|T_f] and V-pair = [T_g|c] are both
# contiguous ascending (c lives in block 4 of the Q tile).
_PERM = np.concatenate([
    np.arange(384, 512),   # o
    np.arange(0, 128),     # i
    np.arange(128, 256),   # f
    np.arange(256, 384),   # g
])
# per-block extra scale for the tanh-half trick: f,i,o rows halved; g not.
_BLK_SCALE = np.concatenate([
    np.full(128, 0.5),     # o
    np.full(128, 0.5),     # i
    np.full(128, 0.5),     # f
    np.full(128, 1.0),     # g
])


def _ensure_ntff_hook():
    """The RL image's antenv lacks axon_hooks; inject it so trace=True works."""
    if "antenv.axon_hooks" in sys.modules:
        return
    mod = types.ModuleType("antenv.axon_hooks")
    mod._hook = None
    mod.set_axon_ntff_profile_hook = lambda h: setattr(mod, "_hook", h)
    mod.get_axon_ntff_profile_hook = lambda: mod._hook
    sys.modules["antenv.axon_hooks"] = mod
    try:
        import antenv
        antenv.axon_hooks = mod
        from trn_agent_boot.trn_boot import _ntff_profile_via_ctypes
        mod.set_axon_ntff_profile_hook(
            _ntff_profile_via_ctypes("/opt/axon/libaxon_pjrt.so"))
    except Exception:
        pass


# ---- gather / zin-unit schedule (first-use ordered) --------------------
# fwd reads zin timestep ti=k at step k (ti 0..S-1); bwd reads
# ti=NTS-1-k at step k (ti NTS-1 .. W).  Block j is first read by fwd at
# step 8j and by bwd at step (NTS-8-8j).  Pieces: quarter q = ti
# [8b+2q, 8b+2q+2), half h = ti [8b+4h, 8b+4h+4), full = whole block.
#
# (An 8-piece bridge via indirect_dma_start was tried and reverted: the
# qPoolDynamic transfers complete no earlier than the SWDGE pipe, and
# their ~1.4us/piece desc-gens delay the library load by 11us.)
#
# W=10 layout: ti 0..83.  fwd reads ti 0..73 (blk0..blk8 + blk9 q0);
# bwd reads ti 83..10 (blk10-h0 [ti 80-83] + blk9 q3..q0 + blk8..blk2 +
# blk1 h1/q1).  SWDGE gather order (piece, block, sub), by first use:
_GATHERS = [
    ("q", 10, 1), ("q", 0, 0), ("q", 10, 0), ("q", 0, 1), ("q", 9, 3),
    ("q", 0, 2), ("q", 9, 2), ("q", 0, 3), ("q", 9, 1),
    ("q", 1, 0), ("q", 9, 0), ("q", 1, 1),
    ("F", 8, 0), ("h", 1, 1),
    ("F", 2, 0), ("F", 7, 0), ("F", 3, 0), ("F", 6, 0),
    ("F", 4, 0), ("F", 5, 0),
]
# zin units: (dir, kind, block, sub, first-use scan step), in use order.
_UNITS = [
    ("f", "q", 0, 0, 0), ("b", "h", 10, 0, 0),
    ("f", "q", 0, 1, 2), ("b", "q", 9, 3, 4),
    ("f", "q", 0, 2, 4), ("b", "q", 9, 2, 6),
    ("f", "q", 0, 3, 6), ("b", "q", 9, 1, 8),
    ("f", "q", 1, 0, 8), ("b", "q", 9, 0, 10),
    ("f", "q", 1, 1, 10), ("b", "F", 8, 0, 12),
    ("f", "h", 1, 1, 12),
    ("f", "F", 2, 0, 16), ("b", "F", 7, 0, 20),
    ("f", "F", 3, 0, 24), ("b", "F", 6, 0, 28),
    ("f", "F", 4, 0, 32), ("b", "F", 5, 0, 36),
    ("f", "F", 5, 0, 40), ("b", "F", 4, 0, 44),
    ("f", "F", 6, 0, 48), ("b", "F", 3, 0, 52),
    ("f", "F", 7, 0, 56), ("b", "F", 2, 0, 60),
    ("f", "F", 8, 0, 64), ("b", "h", 1, 1, 68),
    ("f", "q", 9, 0, 72), ("b", "q", 1, 1, 72),
]
# core-edge warmup pieces (zero bias on the edge core): fwd warmup is
# ti 0..W-1 = blk0 + blk1 q0; bwd warmup ti NTS-W..NTS-1 = blk10-h0 +
# blk9 q3/q2/q1.
def _warm(d, kind, b_, sub):
    if d == "f":
        return b_ == 0 or (b_ == 1 and kind == "q" and sub == 0)
    return b_ == 10 or (b_ == 9 and kind == "q" and sub >= 1)


def build(steps: int = S, nts: int = NTS):
    """Build + compile the per-core Bass program."""
    nb = nts // TPB
    ntok = nts * B
    ch = steps - W
    nc = bacc.Bacc("TRN2", target_bir_lowering=False, debug=False)

    # ---- DRAM I/O ----
    emb_d = nc.dram_tensor("emb", [VPAD, EMB], F16, kind="ExternalInput")
    idx_d = nc.dram_tensor("idx", [128, ntok // 16], I16, kind="ExternalInput")
    wihT_d = {d: nc.dram_tensor(f"wihT_{d}", [EMB, G4], F16, kind="ExternalInput")
              for d in "fb"}
    whhT_d = {d: nc.dram_tensor(f"whhT_{d}", [HD, G4], F16, kind="ExternalInput")
              for d in "fb"}
    # main bias + warmup-window bias (zeroed on edge cores), [128, 4] f32
    bias_d = {d: nc.dram_tensor(f"bias_{d}", [128, 4], F32, kind="ExternalInput")
              for d in "fb"}
    biasw_d = {d: nc.dram_tensor(f"biasw_{d}", [128, 4], F32, kind="ExternalInput")
               for d in "fb"}
    woutT_d = nc.dram_tensor("woutT", [2, HD, TAGS], F16, kind="ExternalInput")
    trans_d = nc.dram_tensor("trans", [128, TAGS * TAGS], F32, kind="ExternalInput")
    ident_d = nc.dram_tensor("ident", [128, 128], F16, kind="ExternalInput")
    crf_d = nc.dram_tensor("crf", [ch * B, TAGS * TAGS], F32, kind="ExternalOutput")

    with tile.TileContext(nc) as tc, ExitStack() as ctx:
        nc.gpsimd.load_library(library_config.mlp)
        const = ctx.enter_context(tc.tile_pool(name="const", bufs=1))
        big = ctx.enter_context(tc.tile_pool(name="big", bufs=1))
        # emission pools opened early so their PSUM banks / SBUF slots are
        # disjoint from the scan pools (no pool-release serialization).
        epsum = ctx.enter_context(tc.tile_pool(name="epsum", bufs=2, space="PSUM"))
        ecrf = ctx.enter_context(tc.tile_pool(name="ecrf", bufs=8))

        # ---- persistent SBUF ----
        idx_sb = const.tile([128, ntok // 16], I16)
        wihT = {d: const.tile([128, 2, G4], F16, tag=f"wihT{d}", name=f"wihT{d}") for d in "fb"}
        whhT = {d: const.tile([HD, G4], F16, tag=f"whhT{d}", name=f"whhT{d}") for d in "fb"}
        bias = {d: const.tile([128, 4], F32, tag=f"bias{d}", name=f"bias{d}") for d in "fb"}
        biasw = {d: const.tile([128, 4], F32, tag=f"biasw{d}", name=f"biasw{d}") for d in "fb"}
        woutT = const.tile([HD, 2, TAGS], F16)
        trans = const.tile([128, TAGS, TAGS], F32)
        ident = const.tile([128, 128], F16)

        # token embeddings, transposed: [128 emb-part, block, emb-half, tok]
        xT = big.tile([128, nb, 2, 512], F16, tag="xT")
        # piece-granular staging (gather out APs must be contiguous, so
        # quarters/halves get their own slots)
        xq = big.tile([128, 12, 2, 128], F16, tag="xq")
        xh = big.tile([128, 2, 2, 256], F16, tag="xh")
        # input projections, [128 gate-part, t, block(oifg), batch]
        zin = {d: big.tile([128, nts, 4, B], F16, tag=f"zin{d}", name=f"zin{d}")
               for d in "fb"}
        # h' histories (fp16), split into 16-col segments so the emission
        # epilogue's tile-granular deps bind to a segment (overlaps the
        # scan) instead of the whole history.
        # fwd: col k+1 = h' after fwd step k; real time t0+i at col W+1+i.
        # bwd: step j writes col steps-j (descending); real time t0+i at
        #   col i+1; col steps+1 is the zero init.
        # Segments: [0..W], then 16-col segments covering the real cols.
        hbnd = [0, W + 1] + [W + 1 + 16 * i for i in range(1, ch // 16 + 1)] \
            + [steps + 2]
        hseg = {d: [big.tile([128, hbnd[i + 1] - hbnd[i], B], F16,
                             tag=f"h{d}{i}", name=f"h{d}{i}")
                    for i in range(len(hbnd) - 1)] for d in "fb"}

        def hcol(d, col):
            for i in range(len(hbnd) - 1):
                if col < hbnd[i + 1]:
                    return hseg[d][i][:, col - hbnd[i], :]
            raise AssertionError(col)

        def hspan2(d, c0):
            """[128, 2, B] span over cols c0, c0+1 (same segment)."""
            for i in range(len(hbnd) - 1):
                if c0 < hbnd[i + 1]:
                    assert c0 + 2 <= hbnd[i + 1], (d, c0)
                    return hseg[d][i][:, c0 - hbnd[i]:c0 - hbnd[i] + 2, :]
            raise AssertionError(c0)

        # ---- load inputs: idx first (feeds the gathers); weight/const
        # DMAs follow.
        nc.sync.dma_start(idx_sb[:], idx_d[:])
        nc.vector.memset(hcol("f", 0), 0.0)
        nc.vector.memset(hcol("b", steps + 1), 0.0)

        def load_consts():
            # ident first: it feeds the PE warm-up matmuls below
            nc.sync.dma_start(ident[:], ident_d[:])
            for d in "fb":
                nc.sync.dma_start(wihT[d][:], wihT_d[d].rearrange("(k p) g -> p k g", p=128))
                nc.sync.dma_start(whhT[d][:], whhT_d[d][:])
                nc.sync.dma_start(bias[d][:], bias_d[d][:])
                nc.sync.dma_start(biasw[d][:], biasw_d[d][:])
            nc.sync.dma_start(woutT[:], woutT_d.rearrange("c h t -> h c t"))
            nc.sync.dma_start(trans[:], trans_d.rearrange("p (i j) -> p i j", i=TAGS))

        # ---- embedding gathers, strictly in scan first-use order ----
        # Pipe-model completion times (us) for wait-pinning: the scheduler's
        # cost model underestimates SWDGE transfers ~2.5x, so gather-dependent
        # work would otherwise be placed too early in the in-order engine
        # streams and stall the scan on real HW.  lib load ~17.5us, then
        # 1.75us/quarter, 2.75us/half, 4.76us/full through the single queue.
        _GDUR = {"q": 1.75, "h": 2.75, "F": 4.76}
        gather_end = {}
        t_pipe = 17.5
        for g in _GATHERS:
            t_pipe += _GDUR[g[0]]
            gather_end[g] = t_pipe

        qslot = {}   # (block, q) -> xq slot
        hslot = {}   # (block, h) -> xh slot
        for g in _GATHERS:
            kind, b_, sub = g
            if kind == "q":
                qi = len(qslot)
                qslot[(b_, sub)] = qi
                nc.gpsimd.dma_gather(
                    xq[:, qi, :, :], emb_d[:, :],
                    idx_sb[:, 32 * b_ + 8 * sub:32 * b_ + 8 * (sub + 1)],
                    128, 128, EMB, transpose=True)
            elif kind == "h":
                hi = len(hslot)
                hslot[(b_, sub)] = hi
                nc.gpsimd.dma_gather(
                    xh[:, hi, :, :], emb_d[:, :],
                    idx_sb[:, 32 * b_ + 16 * sub:32 * b_ + 16 * (sub + 1)],
                    256, 256, EMB, transpose=True)
            else:
                nc.gpsimd.dma_gather(
                    xT[:, b_, :, :], emb_d[:, :],
                    idx_sb[:, 32 * b_:32 * (b_ + 1)],
                    512, 512, EMB, transpose=True)
            if g == _GATHERS[1]:
                # first two (step-0) pieces queued; now queue the SP-side
                # const DMAs so they land while the gathers run.
                load_consts()

        # ---- input projections: zin = x @ Wih.T + b ----
        zpsum = ctx.enter_context(tc.tile_pool(name="zpsum", bufs=2, space="PSUM"))
        _flip = [0]

        def _unit(d, kind, b_, sub):
            """One (dir, piece, gate-slice c x4) GEMM + bias copyback,
            pinned to the pipe-model time its gather really completes."""
            bsel = biasw[d] if _warm(d, kind, b_, sub) else bias[d]
            if kind == "q":
                src = xq[:, qslot[(b_, sub)], :, :]
                ncols, t0, nt = 128, TPB * b_ + 2 * sub, 2
            elif kind == "h":
                src = xh[:, hslot[(b_, sub)], :, :]
                ncols, t0, nt = 256, TPB * b_ + 4 * sub, 4
            else:
                src = xT[:, b_, :, :]
                ncols, t0, nt = 512, TPB * b_, 8
            for c in range(4):
                zp = zpsum.tile([128, 512], F32, tag="zp")
                nc.tensor.matmul(
                    zp[:, :ncols], wihT[d][:, 0, 128 * c:128 * (c + 1)],
                    src[:, 0, :], start=True, stop=False)
                nc.tensor.matmul(
                    zp[:, :ncols], wihT[d][:, 1, 128 * c:128 * (c + 1)],
                    src[:, 1, :], start=False, stop=True)
                dst = zin[d][:, t0:t0 + nt, c, :]
                if _flip[0] % 2 == 0:
                    nc.scalar.activation(dst, zp[:, :ncols], AF.Identity,
                                         bias=bsel[:, c:c + 1])
                else:
                    nc.vector.tensor_scalar(dst, zp[:, :ncols], bsel[:, c:c + 1],
                                            None, ALU.add)
                _flip[0] += 1

        # PE HAM warm-up: the PE would otherwise idle through the ~17us
        # gpsimd ucode load and enter the first GEMMs / scan steps at the
        # cold 1.2 GHz clock.  Dummy ident@ident matmuls during that dead
        # window push it to 8/8 (2.4 GHz) before real work arrives.
        for g_ in range(5):
            wz = zpsum.tile([128, 128], F32, tag="zp", name="wz")
            for i in range(8):
                nc.tensor.matmul(wz[:], ident[:], ident[:],
                                 start=(i == 0), stop=(i == 7))

        # Pin each unit to max(its gather's completion, a need-based time):
        # deferring far-future units spreads their PE/copyback work across
        # the scan's idle slack instead of overloading the first ~20 steps.
        _T0, _PER, _LEAD = 26.0, 2.214, 12.0
        for d, kind, b_, sub, s_use in _UNITS:
            t_ready = max(gather_end[(kind, b_, sub)] + 0.3,
                          _T0 + _PER * s_use - _LEAD)
            with tc.tile_wait_until(t_ready / 1000.0):
                _unit(d, kind, b_, sub)

        # ---- the recurrent scan (fwd + bwd interleaved) ----
        with tc.tile_pool(name="spsum", bufs=4, space="PSUM") as spsum, \
             tc.tile_pool(name="sQ", bufs=10) as sQ, \
             tc.tile_pool(name="sP", bufs=10) as sP, \
             tc.tile_pool(name="sT", bufs=8) as sT:

            def new_z(k):
                """Fresh psum tiles for step k with zin injected (identity mm).
                Emitted one step ahead so gate mms fire as soon as h lands."""
                zt = {}
                for d in "fb":
                    ti = k if d == "f" else nts - 1 - k
                    zt[d] = spsum.tile([128, 4, B], F32, tag="z", name=f"z{d}")
                    nc.tensor.matmul(zt[d][:], ident[:],
                                     zin[d][:, ti, :, :],
                                     start=True, stop=False)
                return zt

            q = {d: sQ.tile([128, 5, B], F32, tag="q", name=f"q{d}") for d in "fb"}
            for d in "fb":
                nc.vector.memset(q[d][:, 4, :], 0.0)
            z = new_z(0)
            for k in range(steps):
                for d, rd_col in (("f", k), ("b", steps + 1 - k)):
                    for c in range(4):
                        nc.tensor.matmul(
                            z[d][:, c, :],
                            whhT[d][:, 128 * c:128 * (c + 1)],
                            hcol(d, rd_col),
                            start=False, stop=(c == 3))
                z_cur, z = z, (new_z(k + 1) if k + 1 < steps else None)
                qn = {d: sQ.tile([128, 5, B], F32, tag="q", name=f"q{d}")
                      for d in "fb"}
                # Stage-interleaved emission: both dirs' ops alternate at
                # each chain stage so the engine queues lock the two chains
                # half a period out of phase.
                wr_col = {"f": k + 1, "b": steps - k}
                P = {}
                for d in "fb":
                    # T = tanh(z') into blocks [o,i,f,g]; state cc = 2*c
                    # sits in block 4.
                    nc.scalar.activation(q[d][:, 0:4, :], z_cur[d][:],
                                         AF.Tanh)
                for d in "fb":
                    # P = [(T_i+1)*T_g | (T_f+1)*cc]
                    P[d] = sP.tile([128, 2, B], F32, tag="P", name="P")
                    nc.vector.scalar_tensor_tensor(
                        P[d][:], q[d][:, 1:3, :], 1.0, q[d][:, 3:5, :],
                        ALU.add, ALU.mult)
                for d in "fb":
                    # cc_new = 2*c_new = 0.5*P1 + P0, written straight into
                    # the next step's Q tile (no separate state-fix op).
                    nc.vector.scalar_tensor_tensor(
                        qn[d][:, 4, :], P[d][:, 1, :], 0.5, P[d][:, 0, :],
                        ALU.mult, ALU.add)
                TC = {}
                for d in "fb":
                    TC[d] = sT.tile([128, B], F32, tag="TC", name="TC")
                    nc.scalar.activation(TC[d][:], qn[d][:, 4, :], AF.Tanh,
                                         scale=0.5)
                for d in "fb":
                    # h' = (T_o + 1) * TC
                    nc.vector.scalar_tensor_tensor(
                        hcol(d, wr_col[d]), q[d][:, 0, :], 1.0, TC[d][:],
                        ALU.add, ALU.mult)
                q = qn

        # ---- emission + CRF broadcast-add + store ----
        # chunk n covers local times 2n, 2n+1 (128 tokens);
        # hf cols W+1+2n..W+2+2n, hb cols 2n+1..2n+2.
        # (256-token chunks were tried: the bigger DVE op concentrates the
        # tail intrusions and lengthens the post-scan chain — net worse.)
        nchunks = ch // 2
        order = sorted(range(nchunks),
                       key=lambda n: max(W + 2 + 2 * n, steps - 1 - 2 * n))
        for n in order:
            e = epsum.tile([128, TAGS], F32, tag="e")
            nc.tensor.matmul(e[:], hspan2("f", W + 1 + 2 * n),
                             woutT[:, 0, :], start=True, stop=False)
            nc.tensor.matmul(e[:], hspan2("b", 1 + 2 * n),
                             woutT[:, 1, :], start=False, stop=True)
            crf_sb = ecrf.tile([128, TAGS, TAGS], F32, tag="crf")
            e_b = e[:, None, :].to_broadcast([128, TAGS, TAGS])
            nc.vector.tensor_tensor(crf_sb[:], e_b, trans[:], ALU.add)
            nc.sync.dma_start(crf_d[128 * n:128 * (n + 1), :], crf_sb[:])

    nc.compile()
    _assert_ldw_pairing(nc)
    return nc


def _assert_ldw_pairing(nc):
    """Every non-self-loading matmul must directly follow an InstLdweights
    whose weights AP matches the matmul's weights operand."""
    for f in nc.m.functions:
        for bb in f.blocks:
            prev_pe = None
            for ins in bb.instructions:
                if ins.engine != mybir.EngineType.PE:
                    continue
                if isinstance(ins, mybir.InstMatmult) and ins.ldweights is False:
                    assert isinstance(prev_pe, mybir.InstLdweights), (
                        f"{ins.name}: non-self-loading matmul not preceded by "
                        f"ldweights (got {type(prev_pe).__name__})")
                    assert repr(prev_pe.ins[0]) == repr(ins.ins[1]), (
                        f"{ins.name}: weights mismatch with {prev_pe.name}")
                prev_pe = ins


_CACHE = {}


def _get_nc():
    if "nc" not in _CACHE:
        _CACHE["nc"] = build()
    return _CACHE["nc"]


def _prep_dir(w_ih, w_hh, b):
    """Permute gates to [o,i,f,g]; apply tanh-half trick (f,i,o rows x0.5)
    and h'=2h compensation (all Whh x0.5)."""
    w_ih = np.asarray(w_ih, np.float32)[_PERM] * _BLK_SCALE[:, None]
    w_hh = np.asarray(w_hh, np.float32)[_PERM] * (0.5 * _BLK_SCALE[:, None])
    b = np.asarray(b, np.float32)[_PERM] * _BLK_SCALE
    wihT = np.ascontiguousarray(w_ih.T).astype(np.float16)
    whhT = np.ascontiguousarray(w_hh.T).astype(np.float16)
    bias = np.ascontiguousarray(b.reshape(4, 128).T).astype(np.float32)
    return wihT, whhT, bias


def make_in_maps(sentences, embedding, W_ih_f, W_hh_f, b_f, W_ih_b, W_hh_b,
                 b_b, W_out, b_out, transition):
    emb = np.zeros((VPAD, EMB), np.float32)
    emb[:VOCAB] = np.asarray(embedding, np.float32)
    emb = emb.astype(np.float16)
    wihT_f, whhT_f, bias_f = _prep_dir(W_ih_f, W_hh_f, b_f)
    wihT_b, whhT_b, bias_b = _prep_dir(W_ih_b, W_hh_b, b_b)
    wo = np.asarray(W_out, np.float32) * 0.5   # h' = 2h compensation
    woutT = np.stack([np.ascontiguousarray(wo[:, :128].T),
                      np.ascontiguousarray(wo[:, 128:].T)])
    woutT = woutT.astype(np.float16)  # [2, 128, 16]
    trans_aug = (np.asarray(transition, np.float32)
                 + np.asarray(b_out, np.float32)[None, :]).reshape(-1)  # [256]
    trans_rep = np.ascontiguousarray(
        np.broadcast_to(trans_aug, (128, 256))).astype(np.float32)
    ident = np.eye(128, dtype=np.float16)
    zeros4 = np.zeros((128, 4), np.float32)

    # tokens per core: times [64c - W, 64c + 64 + W), batch-inner (t, b)
    # order; out-of-range times -> the zero embedding row (ZTOK).
    sent = np.asarray(sentences).astype(np.int64)  # [B, T]
    in_maps = []
    for c in range(NCORES):
        t_lo = CH * c - W
        times = np.arange(t_lo, t_lo + NTS)
        cols = np.clip(times, 0, T - 1)
        toks = sent[:, cols].T.copy()          # [NTS, B]
        toks[(times < 0) | (times >= T)] = ZTOK
        toks = toks.reshape(-1)                # (t, b) order, [NTOK]
        idx = np.tile(toks.reshape(NTOK // 16, 16).T.astype(np.int16), (8, 1))
        in_maps.append({
            "emb": emb, "idx": idx,
            "wihT_f": wihT_f, "wihT_b": wihT_b,
            "whhT_f": whhT_f, "whhT_b": whhT_b,
            "bias_f": bias_f, "bias_b": bias_b,
            "biasw_f": zeros4 if c == 0 else bias_f,
            "biasw_b": zeros4 if c == NCORES - 1 else bias_b,
            "woutT": woutT, "trans": trans_rep, "ident": ident,
        })
    return in_maps


def assemble_out(results):
    out = np.empty((B, T, TAGS, TAGS), np.float32)
    for c in range(NCORES):
        crf = results[c]["crf"].reshape(CH, B, TAGS, TAGS)
        out[:, CH * c:CH * (c + 1)] = crf.transpose(1, 0, 2, 3)
    return out


def kernel(**inputs):
    _ensure_ntff_hook()
    nc = _get_nc()
    in_maps = make_in_maps(**inputs)
    res = run_bass_kernel_spmd(nc, in_maps, list(range(NCORES)))
    return assemble_out(res.results)


# revision 37
# speedup vs baseline: 1.0864x; 1.0864x over previous
"""BiLSTM-CRF Trainium2 kernel — 8-core TIME-chunked scan.

Contract: kernel(**inputs) takes the FULL unsharded inputs (numpy arrays,
keyed as in reference.setup_inputs()) and returns the FULL [B, T, TAGS, TAGS]
crf_scores array.

Sharding: the 512-step scan is latency-bound (per-step serial chain of
~2.2us across PE->ACT->VEC->VEC->ACT->VEC), and per-instruction costs are
almost entirely fixed overhead, so batching all 64 sequences into one op
costs barely more than 8.  We therefore split TIME, not batch: core c
computes time chunk [64c, 64c+64) for the full batch, running its fwd/bwd
scans W extra warmup steps from zero state.  The LSTM here is strongly
contractive (forget gate ~0.5), so the warmup truncation error decays
~0.6^W.  All 16-bit tensors are fp16 (not bf16): same PE throughput,
~8x less quantization noise, which pays for W=12 (vs 16) warmup:
measured rel-err 5.2e-3 (fp16,W=12) vs 1.49e-2 (bf16,W=16).

Cell math is reformulated tanh-only (sigma(x) = (tanh(x/2)+1)/2, with
weights pre-scaled on host, hidden state carried as h' = 2h and cell
state as cc = 2c):
    T = tanh(z')            z' blocks [o, i, f, g] with f,i,o halved
    P = [(T_i+1)*T_g | (T_f+1)*cc]      (one scalar_tensor_tensor)
    cc_new = 0.5*P_f + P_i              (one scalar_tensor_tensor,
                                         written into next step's Q tile)
    TC = tanh(0.5*cc_new)               (ACT free scale)
    h' = (T_o+1)*TC                     (one scalar_tensor_tensor)

Embedding gather: one SWDGE queue only (TRN2), ~725ns + 7.9ns/token per
gather, so the gathers are ordered strictly by the scan's first-use time
(fwd consumes blocks ascending, bwd descending, simultaneously), with
fine pieces (128-token quarters) at the start so the scan can begin
~19us in, coarsening to 256/512-token pieces once the scan is underway.
"""
import sys
import types
from contextlib import ExitStack

import numpy as np

import concourse.bacc as bacc
import concourse.bass as bass
import concourse.mybir as mybir
import concourse.tile as tile
from concourse import library_config
from concourse.bass_utils import run_bass_kernel_spmd

# ---- problem dims (hardcoded per spec) ----
VOCAB = 30000
VPAD = 30720      # emb table padded; row 30000 is all-zeros (warmup token)
ZTOK = 30000
EMB = 256
HD = 128          # per-direction hidden
G4 = 512          # 4*HD gates
TAGS = 16
B, T = 64, 512
NCORES = 8

CH = 64           # time chunk per core
W = 10            # warmup steps
S = CH + W        # scan steps per direction (74)
NTS = CH + 2 * W  # timesteps of tokens/zin per core (84)
NTOK = NTS * B    # tokens per core
TPB = 8           # timesteps per 512-token block
NB = NTS // TPB   # full gather/gemm blocks (10); ti 80-83 is a half-block

F16 = mybir.dt.float16
F32 = mybir.dt.float32
I16 = mybir.dt.int16
AF = mybir.ActivationFunctionType
ALU = mybir.AluOpType

# gate order in reference (jnp.split): i, f, g, o.  Device block order is
# [o, i, f, g] so that W-pair = [T_i|T_f] and V-pair = [T_g|c] are both
# contiguous ascending (c lives in block 4 of the Q tile).
_PERM = np.concatenate([
    np.arange(384, 512),   # o
    np.arange(0, 128),     # i
    np.arange(128, 256),   # f
    np.arange(256, 384),   # g
])
# per-block extra scale for the tanh-half trick: f,i,o rows halved; g not.
_BLK_SCALE = np.concatenate([
    np.full(128, 0.5),     # o
    np.full(128, 0.5),     # i
    np.full(128, 0.5),     # f
    np.full(128, 1.0),     # g
])


def _ensure_ntff_hook():
    """The RL image's antenv lacks axon_hooks; inject it so trace=True works."""
    if "antenv.axon_hooks" in sys.modules:
        return
    mod = types.ModuleType("antenv.axon_hooks")
    mod._hook = None
    mod.set_axon_ntff_profile_hook = lambda h: setattr(mod, "_hook", h)
    mod.get_axon_ntff_profile_hook = lambda: mod._hook
    sys.modules["antenv.axon_hooks"] = mod
    try:
        import antenv
        antenv.axon_hooks = mod
        from trn_agent_boot.trn_boot import _ntff_profile_via_ctypes
        mod.set_axon_ntff_profile_hook(
            _ntff_profile_via_ctypes("/opt/axon/libaxon_pjrt.so"))
    except Exception:
        pass


# ---- gather / zin-unit schedule (first-use ordered) --------------------
# fwd reads zin timestep ti=k at step k (ti 0..S-1); bwd reads
# ti=NTS-1-k at step k (ti NTS-1 .. W).  Block j is first read by fwd at
# step 8j and by bwd at step (NTS-8-8j).  Pieces: quarter q = ti
# [8b+2q, 8b+2q+2), half h = ti [8b+4h, 8b+4h+4), full = whole block.
#
# (An 8-piece bridge via indirect_dma_start was tried and reverted: the
# qPoolDynamic transfers complete no earlier than the SWDGE pipe, and
# their ~1.4us/piece desc-gens delay the library load by 11us.)
#
# W=10 layout: ti 0..83.  fwd reads ti 0..73 (blk0..blk8 + blk9 q0);
# bwd reads ti 83..10 (blk10-h0 [ti 80-83] + blk9 q3..q0 + blk8..blk2 +
# blk1 h1/q1).  SWDGE gather order (piece, block, sub), by first use:
_GATHERS = [
    ("q", 10, 1), ("q", 0, 0), ("q", 10, 0), ("q", 0, 1), ("q", 9, 3),
    ("q", 0, 2), ("q", 9, 2), ("q", 0, 3), ("q", 9, 1),
    ("q", 1, 0), ("q", 9, 0), ("q", 1, 1),
    ("F", 8, 0), ("h", 1, 1),
    ("F", 2, 0), ("F", 7, 0), ("F", 3, 0), ("F", 6, 0),
    ("F", 4, 0), ("F", 5, 0),
]
# zin units: (dir, kind, block, sub, first-use scan step), in use order.
_UNITS = [
    ("f", "q", 0, 0, 0), ("b", "q", 10, 1, 0),
    ("b", "q", 10, 0, 2),
    ("f", "q", 0, 1, 2), ("b", "q", 9, 3, 4),
    ("f", "q", 0, 2, 4), ("b", "q", 9, 2, 6),
    ("f", "q", 0, 3, 6), ("b", "q", 9, 1, 8),
    ("f", "q", 1, 0, 8), ("b", "q", 9, 0, 10),
    ("f", "q", 1, 1, 10), ("b", "F", 8, 0, 12),
    ("f", "h", 1, 1, 12),
    ("f", "F", 2, 0, 16), ("b", "F", 7, 0, 20),
    ("f", "F", 3, 0, 24), ("b", "F", 6, 0, 28),
    ("f", "F", 4, 0, 32), ("b", "F", 5, 0, 36),
    ("f", "F", 5, 0, 40), ("b", "F", 4, 0, 44),
    ("f", "F", 6, 0, 48), ("b", "F", 3, 0, 52),
    ("f", "F", 7, 0, 56), ("b", "F", 2, 0, 60),
    ("f", "F", 8, 0, 64), ("b", "h", 1, 1, 68),
    ("f", "q", 9, 0, 72), ("b", "q", 1, 1, 72),
]
# core-edge warmup pieces (zero bias on the edge core): fwd warmup is
# ti 0..W-1 = blk0 + blk1 q0; bwd warmup ti NTS-W..NTS-1 = blk10-h0 +
# blk9 q3/q2/q1.
def _warm(d, kind, b_, sub):
    if d == "f":
        return b_ == 0 or (b_ == 1 and kind == "q" and sub == 0)
    return b_ == 10 or (b_ == 9 and kind == "q" and sub >= 1)


def build(steps: int = S, nts: int = NTS):
    """Build + compile the per-core Bass program."""
    nb = nts // TPB
    ntok = nts * B
    ch = steps - W
    nc = bacc.Bacc("TRN2", target_bir_lowering=False, debug=False)

    # ---- DRAM I/O ----
    emb_d = nc.dram_tensor("emb", [VPAD, EMB], F16, kind="ExternalInput")
    idx_d = nc.dram_tensor("idx", [128, ntok // 16], I16, kind="ExternalInput")
    wihT_d = {d: nc.dram_tensor(f"wihT_{d}", [EMB, G4], F16, kind="ExternalInput")
              for d in "fb"}
    whhT_d = {d: nc.dram_tensor(f"whhT_{d}", [HD, G4], F16, kind="ExternalInput")
              for d in "fb"}
    # main bias + warmup-window bias (zeroed on edge cores), [128, 4] f32
    bias_d = {d: nc.dram_tensor(f"bias_{d}", [128, 4], F32, kind="ExternalInput")
              for d in "fb"}
    biasw_d = {d: nc.dram_tensor(f"biasw_{d}", [128, 4], F32, kind="ExternalInput")
               for d in "fb"}
    woutT_d = nc.dram_tensor("woutT", [2, HD, TAGS], F16, kind="ExternalInput")
    trans_d = nc.dram_tensor("trans", [128, TAGS * TAGS], F32, kind="ExternalInput")
    ident_d = nc.dram_tensor("ident", [128, 128], F16, kind="ExternalInput")
    crf_d = nc.dram_tensor("crf", [ch * B, TAGS * TAGS], F32, kind="ExternalOutput")

    with tile.TileContext(nc) as tc, ExitStack() as ctx:
        nc.gpsimd.load_library(library_config.mlp)
        const = ctx.enter_context(tc.tile_pool(name="const", bufs=1))
        big = ctx.enter_context(tc.tile_pool(name="big", bufs=1))
        # emission pools opened early so their PSUM banks / SBUF slots are
        # disjoint from the scan pools (no pool-release serialization).
        epsum = ctx.enter_context(tc.tile_pool(name="epsum", bufs=2, space="PSUM"))
        ecrf = ctx.enter_context(tc.tile_pool(name="ecrf", bufs=8))

        # ---- persistent SBUF ----
        idx_sb = const.tile([128, ntok // 16], I16)
        wihT = {d: const.tile([128, 2, G4], F16, tag=f"wihT{d}", name=f"wihT{d}") for d in "fb"}
        whhT = {d: const.tile([HD, G4], F16, tag=f"whhT{d}", name=f"whhT{d}") for d in "fb"}
        bias = {d: const.tile([128, 4], F32, tag=f"bias{d}", name=f"bias{d}") for d in "fb"}
        biasw = {d: const.tile([128, 4], F32, tag=f"biasw{d}", name=f"biasw{d}") for d in "fb"}
        woutT = const.tile([HD, 2, TAGS], F16)
        trans = const.tile([128, TAGS, TAGS], F32)
        ident = const.tile([128, 128], F16)

        # token embeddings, transposed: [128 emb-part, block, emb-half, tok]
        xT = big.tile([128, nb, 2, 512], F16, tag="xT")
        # piece-granular staging (gather out APs must be contiguous, so
        # quarters/halves get their own slots)
        xq = big.tile([128, 12, 2, 128], F16, tag="xq")
        xh = big.tile([128, 2, 2, 256], F16, tag="xh")
        # input projections, [128 gate-part, t, block(oifg), batch]
        zin = {d: big.tile([128, nts, 4, B], F16, tag=f"zin{d}", name=f"zin{d}")
               for d in "fb"}
        # h' histories (fp16), split into 16-col segments so the emission
        # epilogue's tile-granular deps bind to a segment (overlaps the
        # scan) instead of the whole history.
        # fwd: col k+1 = h' after fwd step k; real time t0+i at col W+1+i.
        # bwd: step j writes col steps-j (descending); real time t0+i at
        #   col i+1; col steps+1 is the zero init.
        # Segments: [0..W], then 16-col segments covering the real cols.
        hbnd = [0, W + 1] + [W + 1 + 16 * i for i in range(1, ch // 16 + 1)] \
            + [steps + 2]
        hseg = {d: [big.tile([128, hbnd[i + 1] - hbnd[i], B], F16,
                             tag=f"h{d}{i}", name=f"h{d}{i}")
                    for i in range(len(hbnd) - 1)] for d in "fb"}

        def hcol(d, col):
            for i in range(len(hbnd) - 1):
                if col < hbnd[i + 1]:
                    return hseg[d][i][:, col - hbnd[i], :]
            raise AssertionError(col)

        def hspan2(d, c0):
            """[128, 2, B] span over cols c0, c0+1 (same segment)."""
            for i in range(len(hbnd) - 1):
                if c0 < hbnd[i + 1]:
                    assert c0 + 2 <= hbnd[i + 1], (d, c0)
                    return hseg[d][i][:, c0 - hbnd[i]:c0 - hbnd[i] + 2, :]
            raise AssertionError(c0)

        # ---- load inputs: idx first (feeds the gathers); weight/const
        # DMAs follow.
        nc.sync.dma_start(idx_sb[:], idx_d[:])
        nc.vector.memset(hcol("f", 0), 0.0)
        nc.vector.memset(hcol("b", steps + 1), 0.0)

        def load_consts():
            # ident first: it feeds the PE warm-up matmuls below
            nc.sync.dma_start(ident[:], ident_d[:])
            for d in "fb":
                nc.sync.dma_start(wihT[d][:], wihT_d[d].rearrange("(k p) g -> p k g", p=128))
                nc.sync.dma_start(whhT[d][:], whhT_d[d][:])
                nc.sync.dma_start(bias[d][:], bias_d[d][:])
                nc.sync.dma_start(biasw[d][:], biasw_d[d][:])
            nc.sync.dma_start(woutT[:], woutT_d.rearrange("c h t -> h c t"))
            nc.sync.dma_start(trans[:], trans_d.rearrange("p (i j) -> p i j", i=TAGS))

        # ---- embedding gathers, strictly in scan first-use order ----
        # Pipe-model completion times (us) for wait-pinning: the scheduler's
        # cost model underestimates SWDGE transfers ~2.5x, so gather-dependent
        # work would otherwise be placed too early in the in-order engine
        # streams and stall the scan on real HW.  lib load ~17.5us, then
        # 1.75us/quarter, 2.75us/half, 4.76us/full through the single queue.
        _GDUR = {"q": 1.75, "h": 2.75, "F": 4.76}
        gather_end = {}
        t_pipe = 17.5
        for g in _GATHERS:
            t_pipe += _GDUR[g[0]]
            gather_end[g] = t_pipe

        qslot = {}   # (block, q) -> xq slot
        hslot = {}   # (block, h) -> xh slot
        for g in _GATHERS:
            kind, b_, sub = g
            if kind == "q":
                qi = len(qslot)
                qslot[(b_, sub)] = qi
                nc.gpsimd.dma_gather(
                    xq[:, qi, :, :], emb_d[:, :],
                    idx_sb[:, 32 * b_ + 8 * sub:32 * b_ + 8 * (sub + 1)],
                    128, 128, EMB, transpose=True)
            elif kind == "h":
                hi = len(hslot)
                hslot[(b_, sub)] = hi
                nc.gpsimd.dma_gather(
                    xh[:, hi, :, :], emb_d[:, :],
                    idx_sb[:, 32 * b_ + 16 * sub:32 * b_ + 16 * (sub + 1)],
                    256, 256, EMB, transpose=True)
            else:
                nc.gpsimd.dma_gather(
                    xT[:, b_, :, :], emb_d[:, :],
                    idx_sb[:, 32 * b_:32 * (b_ + 1)],
                    512, 512, EMB, transpose=True)
            if g == _GATHERS[1]:
                # first two (step-0) pieces queued; now queue the SP-side
                # const DMAs so they land while the gathers run.
                load_consts()

        # ---- input projections: zin = x @ Wih.T + b ----
        zpsum = ctx.enter_context(tc.tile_pool(name="zpsum", bufs=2, space="PSUM"))
        _flip = [0]

        def _unit(d, kind, b_, sub):
            """One (dir, piece, gate-slice c x4) GEMM + bias copyback,
            pinned to the pipe-model time its gather really completes."""
            bsel = biasw[d] if _warm(d, kind, b_, sub) else bias[d]
            if kind == "q":
                src = xq[:, qslot[(b_, sub)], :, :]
                ncols, t0, nt = 128, TPB * b_ + 2 * sub, 2
            elif kind == "h":
                src = xh[:, hslot[(b_, sub)], :, :]
                ncols, t0, nt = 256, TPB * b_ + 4 * sub, 4
            else:
                src = xT[:, b_, :, :]
                ncols, t0, nt = 512, TPB * b_, 8
            for c in range(4):
                zp = zpsum.tile([128, 512], F32, tag="zp")
                nc.tensor.matmul(
                    zp[:, :ncols], wihT[d][:, 0, 128 * c:128 * (c + 1)],
                    src[:, 0, :], start=True, stop=False)
                nc.tensor.matmul(
                    zp[:, :ncols], wihT[d][:, 1, 128 * c:128 * (c + 1)],
                    src[:, 1, :], start=False, stop=True)
                dst = zin[d][:, t0:t0 + nt, c, :]
                if _flip[0] % 2 == 0:
                    nc.scalar.activation(dst, zp[:, :ncols], AF.Identity,
                                         bias=bsel[:, c:c + 1])
                else:
                    nc.vector.tensor_scalar(dst, zp[:, :ncols], bsel[:, c:c + 1],
                                            None, ALU.add)
                _flip[0] += 1

        # PE HAM warm-up: the PE would otherwise idle through the ~17us
        # gpsimd ucode load and enter the first GEMMs / scan steps at the
        # cold 1.2 GHz clock.  Dummy ident@ident matmuls during that dead
        # window push it to 8/8 (2.4 GHz) before real work arrives.
        for g_ in range(5):
            wz = zpsum.tile([128, 128], F32, tag="zp", name="wz")
            for i in range(8):
                nc.tensor.matmul(wz[:], ident[:], ident[:],
                                 start=(i == 0), stop=(i == 7))

        # Units with near first-uses are emitted upfront (gather-pinned).
        # Far-future units are emitted INSIDE the scan loop ~10 steps before
        # first use: their emission priority keeps them out of the early
        # steps' engine queues, the gather pin keeps them from scheduling
        # before their data lands, and the scheduler takes the max — so the
        # copyback/GEMM work spreads across the scan's idle slack instead of
        # overloading the first ~20 steps.
        inloop = {}
        for d, kind, b_, sub, s_use in _UNITS:
            if s_use < 16:
                t_ready = gather_end[(kind, b_, sub)] + 0.3
                with tc.tile_wait_until(t_ready / 1000.0):
                    _unit(d, kind, b_, sub)
            else:
                inloop.setdefault(max(0, s_use - 10), []).append(
                    (d, kind, b_, sub))

        def inloop_units(k):
            for d, kind, b_, sub in inloop.get(k, []):
                t_ready = gather_end[(kind, b_, sub)] + 0.3
                with tc.tile_wait_until(t_ready / 1000.0):
                    _unit(d, kind, b_, sub)

        # ---- the recurrent scan (fwd + bwd interleaved) ----
        with tc.tile_pool(name="spsum", bufs=4, space="PSUM") as spsum, \
             tc.tile_pool(name="sQ", bufs=10) as sQ, \
             tc.tile_pool(name="sP", bufs=10) as sP, \
             tc.tile_pool(name="sT", bufs=8) as sT:

            def new_z(k):
                """Fresh psum tiles for step k with zin injected (identity mm).
                Emitted one step ahead so gate mms fire as soon as h lands."""
                zt = {}
                for d in "fb":
                    ti = k if d == "f" else nts - 1 - k
                    zt[d] = spsum.tile([128, 4, B], F32, tag="z", name=f"z{d}")
                    nc.tensor.matmul(zt[d][:], ident[:],
                                     zin[d][:, ti, :, :],
                                     start=True, stop=False)
                return zt

            q = {d: sQ.tile([128, 5, B], F32, tag="q", name=f"q{d}") for d in "fb"}
            for d in "fb":
                nc.vector.memset(q[d][:, 4, :], 0.0)
            z = new_z(0)
            for k in range(steps):
                for d, rd_col in (("f", k), ("b", steps + 1 - k)):
                    for c in range(4):
                        nc.tensor.matmul(
                            z[d][:, c, :],
                            whhT[d][:, 128 * c:128 * (c + 1)],
                            hcol(d, rd_col),
                            start=False, stop=(c == 3))
                z_cur, z = z, (new_z(k + 1) if k + 1 < steps else None)
                qn = {d: sQ.tile([128, 5, B], F32, tag="q", name=f"q{d}")
                      for d in "fb"}
                # Stage-interleaved emission: both dirs' ops alternate at
                # each chain stage so the engine queues lock the two chains
                # half a period out of phase.
                wr_col = {"f": k + 1, "b": steps - k}
                P = {}
                for d in "fb":
                    # T = tanh(z') into blocks [o,i,f,g]; state cc = 2*c
                    # sits in block 4.
                    nc.scalar.activation(q[d][:, 0:4, :], z_cur[d][:],
                                         AF.Tanh)
                for d in "fb":
                    # P = [(T_i+1)*T_g | (T_f+1)*cc]
                    P[d] = sP.tile([128, 2, B], F32, tag="P", name="P")
                    nc.vector.scalar_tensor_tensor(
                        P[d][:], q[d][:, 1:3, :], 1.0, q[d][:, 3:5, :],
                        ALU.add, ALU.mult)
                for d in "fb":
                    # cc_new = 2*c_new = 0.5*P1 + P0, written straight into
                    # the next step's Q tile (no separate state-fix op).
                    nc.vector.scalar_tensor_tensor(
                        qn[d][:, 4, :], P[d][:, 1, :], 0.5, P[d][:, 0, :],
                        ALU.mult, ALU.add)
                TC = {}
                for d in "fb":
                    TC[d] = sT.tile([128, B], F32, tag="TC", name="TC")
                    nc.scalar.activation(TC[d][:], qn[d][:, 4, :], AF.Tanh,
                                         scale=0.5)
                for d in "fb":
                    # h' = (T_o + 1) * TC
                    nc.vector.scalar_tensor_tensor(
                        hcol(d, wr_col[d]), q[d][:, 0, :], 1.0, TC[d][:],
                        ALU.add, ALU.mult)
                q = qn
                inloop_units(k)

        # ---- emission + CRF broadcast-add + store ----
        # chunk n covers local times 2n, 2n+1 (128 tokens);
        # hf cols W+1+2n..W+2+2n, hb cols 2n+1..2n+2.
        # (256-token chunks were tried: the bigger DVE op concentrates the
        # tail intrusions and lengthens the post-scan chain — net worse.)
        nchunks = ch // 2
        order = sorted(range(nchunks),
                       key=lambda n: max(W + 2 + 2 * n, steps - 1 - 2 * n))
        for n in order:
            e = epsum.tile([128, TAGS], F32, tag="e")
            nc.tensor.matmul(e[:], hspan2("f", W + 1 + 2 * n),
                             woutT[:, 0, :], start=True, stop=False)
            nc.tensor.matmul(e[:], hspan2("b", 1 + 2 * n),
                             woutT[:, 1, :], start=False, stop=True)
            crf_sb = ecrf.tile([128, TAGS, TAGS], F32, tag="crf")
            e_b = e[:, None, :].to_broadcast([128, TAGS, TAGS])
            nc.vector.tensor_tensor(crf_sb[:], e_b, trans[:], ALU.add)
            nc.sync.dma_start(crf_d[128 * n:128 * (n + 1), :], crf_sb[:])

    nc.compile()
    _assert_ldw_pairing(nc)
    return nc


def _assert_ldw_pairing(nc):
    """Every non-self-loading matmul must directly follow an InstLdweights
    whose weights AP matches the matmul's weights operand."""
    for f in nc.m.functions:
        for bb in f.blocks:
            prev_pe = None
            for ins in bb.instructions:
                if ins.engine != mybir.EngineType.PE:
                    continue
                if isinstance(ins, mybir.InstMatmult) and ins.ldweights is False:
                    assert isinstance(prev_pe, mybir.InstLdweights), (
                        f"{ins.name}: non-self-loading matmul not preceded by "
                        f"ldweights (got {type(prev_pe).__name__})")
                    assert repr(prev_pe.ins[0]) == repr(ins.ins[1]), (
                        f"{ins.name}: weights mismatch with {prev_pe.name}")
                prev_pe = ins


_CACHE = {}


def _get_nc():
    if "nc" not in _CACHE:
        _CACHE["nc"] = build()
    return _CACHE["nc"]


def _prep_dir(w_ih, w_hh, b):
    """Permute gates to [o,i,f,g]; apply tanh-half trick (f,i,o rows x0.5)
    and h'=2h compensation (all Whh x0.5)."""
    w_ih = np.asarray(w_ih, np.float32)[_PERM] * _BLK_SCALE[:, None]
    w_hh = np.asarray(w_hh, np.float32)[_PERM] * (0.5 * _BLK_SCALE[:, None])
    b = np.asarray(b, np.float32)[_PERM] * _BLK_SCALE
    wihT = np.ascontiguousarray(w_ih.T).astype(np.float16)
    whhT = np.ascontiguousarray(w_hh.T).astype(np.float16)
    bias = np.ascontiguousarray(b.reshape(4, 128).T).astype(np.float32)
    return wihT, whhT, bias


def make_in_maps(sentences, embedding, W_ih_f, W_hh_f, b_f, W_ih_b, W_hh_b,
                 b_b, W_out, b_out, transition):
    emb = np.zeros((VPAD, EMB), np.float32)
    emb[:VOCAB] = np.asarray(embedding, np.float32)
    emb = emb.astype(np.float16)
    wihT_f, whhT_f, bias_f = _prep_dir(W_ih_f, W_hh_f, b_f)
    wihT_b, whhT_b, bias_b = _prep_dir(W_ih_b, W_hh_b, b_b)
    wo = np.asarray(W_out, np.float32) * 0.5   # h' = 2h compensation
    woutT = np.stack([np.ascontiguousarray(wo[:, :128].T),
                      np.ascontiguousarray(wo[:, 128:].T)])
    woutT = woutT.astype(np.float16)  # [2, 128, 16]
    trans_aug = (np.asarray(transition, np.float32)
                 + np.asarray(b_out, np.float32)[None, :]).reshape(-1)  # [256]
    trans_rep = np.ascontiguousarray(
        np.broadcast_to(trans_aug, (128, 256))).astype(np.float32)
    ident = np.eye(128, dtype=np.float16)
    zeros4 = np.zeros((128, 4), np.float32)

    # tokens per core: times [64c - W, 64c + 64 + W), batch-inner (t, b)
    # order; out-of-range times -> the zero embedding row (ZTOK).
    sent = np.asarray(sentences).astype(np.int64)  # [B, T]
    in_maps = []
    for c in range(NCORES):
        t_lo = CH * c - W
        times = np.arange(t_lo, t_lo + NTS)
        cols = np.clip(times, 0, T - 1)
        toks = sent[:, cols].T.copy()          # [NTS, B]
        toks[(times < 0) | (times >= T)] = ZTOK
        toks = toks.reshape(-1)                # (t, b) order, [NTOK]
        idx = np.tile(toks.reshape(NTOK // 16, 16).T.astype(np.int16), (8, 1))
        in_maps.append({
            "emb": emb, "idx": idx,
            "wihT_f": wihT_f, "wihT_b": wihT_b,
            "whhT_f": whhT_f, "whhT_b": whhT_b,
            "bias_f": bias_f, "bias_b": bias_b,
            "biasw_f": zeros4 if c == 0 else bias_f,
            "biasw_b": zeros4 if c == NCORES - 1 else bias_b,
            "woutT": woutT, "trans": trans_rep, "ident": ident,
        })
    return in_maps


def assemble_out(results):
    out = np.empty((B, T, TAGS, TAGS), np.float32)
    for c in range(NCORES):
        crf = results[c]["crf"].reshape(CH, B, TAGS, TAGS)
        out[:, CH * c:CH * (c + 1)] = crf.transpose(1, 0, 2, 3)
    return out


def kernel(**inputs):
    _ensure_ntff_hook()
    nc = _get_nc()
    in_maps = make_in_maps(**inputs)
    res = run_bass_kernel_spmd(nc, in_maps, list(range(NCORES)))
    return assemble_out(res.results)


# revision 39
# speedup vs baseline: 1.1092x; 1.0210x over previous
"""BiLSTM-CRF Trainium2 kernel — 8-core TIME-chunked scan.

Contract: kernel(**inputs) takes the FULL unsharded inputs (numpy arrays,
keyed as in reference.setup_inputs()) and returns the FULL [B, T, TAGS, TAGS]
crf_scores array.

Sharding: the 512-step scan is latency-bound (per-step serial chain of
~2.2us across PE->ACT->VEC->VEC->ACT->VEC), and per-instruction costs are
almost entirely fixed overhead, so batching all 64 sequences into one op
costs barely more than 8.  We therefore split TIME, not batch: core c
computes time chunk [64c, 64c+64) for the full batch, running its fwd/bwd
scans W extra warmup steps from zero state.  The LSTM here is strongly
contractive (forget gate ~0.5), so the warmup truncation error decays
~0.6^W.  All 16-bit tensors are fp16 (not bf16): same PE throughput,
~8x less quantization noise, which pays for W=12 (vs 16) warmup:
measured rel-err 5.2e-3 (fp16,W=12) vs 1.49e-2 (bf16,W=16).

Cell math is reformulated tanh-only (sigma(x) = (tanh(x/2)+1)/2, with
weights pre-scaled on host, hidden state carried as h' = 2h and cell
state as cc = 2c):
    T = tanh(z')            z' blocks [o, i, f, g] with f,i,o halved
    P = [(T_i+1)*T_g | (T_f+1)*cc]      (one scalar_tensor_tensor)
    cc_new = 0.5*P_f + P_i              (one scalar_tensor_tensor,
                                         written into next step's Q tile)
    TC = tanh(0.5*cc_new)               (ACT free scale)
    h' = (T_o+1)*TC                     (one scalar_tensor_tensor)

Embedding gather: one SWDGE queue only (TRN2), ~725ns + 7.9ns/token per
gather, so the gathers are ordered strictly by the scan's first-use time
(fwd consumes blocks ascending, bwd descending, simultaneously), with
fine pieces (128-token quarters) at the start so the scan can begin
~19us in, coarsening to 256/512-token pieces once the scan is underway.
"""
import sys
import types
from contextlib import ExitStack

import numpy as np

import concourse.bacc as bacc
import concourse.bass as bass
import concourse.mybir as mybir
import concourse.tile as tile
from concourse import library_config
from concourse.bass_utils import run_bass_kernel_spmd

# ---- problem dims (hardcoded per spec) ----
VOCAB = 30000
VPAD = 30720      # emb table padded; row 30000 is all-zeros (warmup token)
ZTOK = 30000
EMB = 256
HD = 128          # per-direction hidden
G4 = 512          # 4*HD gates
TAGS = 16
B, T = 64, 512
NCORES = 8

CH = 64           # time chunk per core
W = 10            # warmup steps
S = CH + W        # scan steps per direction (74)
NTS = CH + 2 * W  # timesteps of tokens/zin per core (84)
NTOK = NTS * B    # tokens per core
TPB = 8           # timesteps per 512-token block
NB = NTS // TPB   # full gather/gemm blocks (10); ti 80-83 is a half-block

F16 = mybir.dt.float16
F32 = mybir.dt.float32
I16 = mybir.dt.int16
AF = mybir.ActivationFunctionType
ALU = mybir.AluOpType

# gate order in reference (jnp.split): i, f, g, o.  Device block order is
# [o, i, f, g] so that W-pair = [T_i|T_f] and V-pair = [T_g|c] are both
# contiguous ascending (c lives in block 4 of the Q tile).
_PERM = np.concatenate([
    np.arange(384, 512),   # o
    np.arange(0, 128),     # i
    np.arange(128, 256),   # f
    np.arange(256, 384),   # g
])
# per-block extra scale for the tanh-half trick: f,i,o rows halved; g not.
_BLK_SCALE = np.concatenate([
    np.full(128, 0.5),     # o
    np.full(128, 0.5),     # i
    np.full(128, 0.5),     # f
    np.full(128, 1.0),     # g
])


def _ensure_ntff_hook():
    """The RL image's antenv lacks axon_hooks; inject it so trace=True works."""
    if "antenv.axon_hooks" in sys.modules:
        return
    mod = types.ModuleType("antenv.axon_hooks")
    mod._hook = None
    mod.set_axon_ntff_profile_hook = lambda h: setattr(mod, "_hook", h)
    mod.get_axon_ntff_profile_hook = lambda: mod._hook
    sys.modules["antenv.axon_hooks"] = mod
    try:
        import antenv
        antenv.axon_hooks = mod
        from trn_agent_boot.trn_boot import _ntff_profile_via_ctypes
        mod.set_axon_ntff_profile_hook(
            _ntff_profile_via_ctypes("/opt/axon/libaxon_pjrt.so"))
    except Exception:
        pass


# ---- gather / zin-unit schedule (first-use ordered) --------------------
# fwd reads zin timestep ti=k at step k (ti 0..S-1); bwd reads
# ti=NTS-1-k at step k (ti NTS-1 .. W).  Block j is first read by fwd at
# step 8j and by bwd at step (NTS-8-8j).  Pieces: quarter q = ti
# [8b+2q, 8b+2q+2), half h = ti [8b+4h, 8b+4h+4), full = whole block.
#
# (An 8-piece bridge via indirect_dma_start was tried and reverted: the
# qPoolDynamic transfers complete no earlier than the SWDGE pipe, and
# their ~1.4us/piece desc-gens delay the library load by 11us.)
#
# W=10 layout: ti 0..83.  fwd reads ti 0..73 (blk0..blk8 + blk9 q0);
# bwd reads ti 83..10 (blk10-h0 [ti 80-83] + blk9 q3..q0 + blk8..blk2 +
# blk1 h1/q1).  SWDGE gather order (piece, block, sub), by first use:
_GATHERS = [
    ("q", 10, 1), ("q", 0, 0), ("q", 10, 0), ("q", 0, 1), ("q", 9, 3),
    ("q", 0, 2), ("q", 9, 2), ("q", 0, 3), ("q", 9, 1),
    ("q", 1, 0), ("q", 9, 0), ("q", 1, 1),
    ("F", 8, 0), ("h", 1, 1),
    ("F", 2, 0), ("F", 7, 0), ("F", 3, 0), ("F", 6, 0),
    ("F", 4, 0), ("F", 5, 0),
]
# zin units: (dir, kind, block, sub, first-use scan step), in use order.
_UNITS = [
    ("f", "q", 0, 0, 0), ("b", "q", 10, 1, 0),
    ("b", "q", 10, 0, 2),
    ("f", "q", 0, 1, 2), ("b", "q", 9, 3, 4),
    ("f", "q", 0, 2, 4), ("b", "q", 9, 2, 6),
    ("f", "q", 0, 3, 6), ("b", "q", 9, 1, 8),
    ("f", "q", 1, 0, 8), ("b", "q", 9, 0, 10),
    ("f", "q", 1, 1, 10), ("b", "F", 8, 0, 12),
    ("f", "h", 1, 1, 12),
    ("f", "F", 2, 0, 16), ("b", "F", 7, 0, 20),
    ("f", "F", 3, 0, 24), ("b", "F", 6, 0, 28),
    ("f", "F", 4, 0, 32), ("b", "F", 5, 0, 36),
    ("f", "F", 5, 0, 40), ("b", "F", 4, 0, 44),
    ("f", "F", 6, 0, 48), ("b", "F", 3, 0, 52),
    ("f", "F", 7, 0, 56), ("b", "F", 2, 0, 60),
    ("f", "F", 8, 0, 64), ("b", "h", 1, 1, 68),
    ("f", "q", 9, 0, 72), ("b", "q", 1, 1, 72),
]
# core-edge warmup pieces (zero bias on the edge core): fwd warmup is
# ti 0..W-1 = blk0 + blk1 q0; bwd warmup ti NTS-W..NTS-1 = blk10-h0 +
# blk9 q3/q2/q1.
def _warm(d, kind, b_, sub):
    if d == "f":
        return b_ == 0 or (b_ == 1 and kind == "q" and sub == 0)
    return b_ == 10 or (b_ == 9 and kind == "q" and sub >= 1)


def build(steps: int = S, nts: int = NTS):
    """Build + compile the per-core Bass program."""
    nb = nts // TPB
    ntok = nts * B
    ch = steps - W
    nc = bacc.Bacc("TRN2", target_bir_lowering=False, debug=False)

    # ---- DRAM I/O ----
    emb_d = nc.dram_tensor("emb", [VPAD, EMB], F16, kind="ExternalInput")
    idx_d = nc.dram_tensor("idx", [128, ntok // 16], I16, kind="ExternalInput")
    wihT_d = {d: nc.dram_tensor(f"wihT_{d}", [EMB, G4], F16, kind="ExternalInput")
              for d in "fb"}
    whhT_d = {d: nc.dram_tensor(f"whhT_{d}", [HD, G4], F16, kind="ExternalInput")
              for d in "fb"}
    # main bias + warmup-window bias (zeroed on edge cores), [128, 4] f32
    bias_d = {d: nc.dram_tensor(f"bias_{d}", [128, 4], F32, kind="ExternalInput")
              for d in "fb"}
    biasw_d = {d: nc.dram_tensor(f"biasw_{d}", [128, 4], F32, kind="ExternalInput")
               for d in "fb"}
    woutT_d = nc.dram_tensor("woutT", [2, HD, TAGS], F16, kind="ExternalInput")
    trans_d = nc.dram_tensor("trans", [128, TAGS * TAGS], F32, kind="ExternalInput")
    ident_d = nc.dram_tensor("ident", [128, 128], F16, kind="ExternalInput")
    crf_d = nc.dram_tensor("crf", [ch * B, TAGS * TAGS], F32, kind="ExternalOutput")

    with tile.TileContext(nc) as tc, ExitStack() as ctx:
        nc.gpsimd.load_library(library_config.mlp)
        const = ctx.enter_context(tc.tile_pool(name="const", bufs=1))
        big = ctx.enter_context(tc.tile_pool(name="big", bufs=1))
        # emission pools opened early so their PSUM banks / SBUF slots are
        # disjoint from the scan pools (no pool-release serialization).
        epsum = ctx.enter_context(tc.tile_pool(name="epsum", bufs=2, space="PSUM"))
        ecrf = ctx.enter_context(tc.tile_pool(name="ecrf", bufs=8))

        # ---- persistent SBUF ----
        idx_sb = const.tile([128, ntok // 16], I16)
        wihT = {d: const.tile([128, 2, G4], F16, tag=f"wihT{d}", name=f"wihT{d}") for d in "fb"}
        whhT = {d: const.tile([HD, G4], F16, tag=f"whhT{d}", name=f"whhT{d}") for d in "fb"}
        bias = {d: const.tile([128, 4], F32, tag=f"bias{d}", name=f"bias{d}") for d in "fb"}
        biasw = {d: const.tile([128, 4], F32, tag=f"biasw{d}", name=f"biasw{d}") for d in "fb"}
        woutT = const.tile([HD, 2, TAGS], F16)
        trans = const.tile([128, TAGS, TAGS], F32)
        ident = const.tile([128, 128], F16)

        # token embeddings, transposed: [128 emb-part, block, emb-half, tok]
        xT = big.tile([128, nb, 2, 512], F16, tag="xT")
        # piece-granular staging (gather out APs must be contiguous, so
        # quarters/halves get their own slots)
        xq = big.tile([128, 12, 2, 128], F16, tag="xq")
        xh = big.tile([128, 2, 2, 256], F16, tag="xh")
        # input projections, [128 gate-part, t, block(oifg), batch]
        zin = {d: big.tile([128, nts, 4, B], F16, tag=f"zin{d}", name=f"zin{d}")
               for d in "fb"}
        # h' histories (fp16), split into 16-col segments so the emission
        # epilogue's tile-granular deps bind to a segment (overlaps the
        # scan) instead of the whole history.
        # fwd: col k+1 = h' after fwd step k; real time t0+i at col W+1+i.
        # bwd: step j writes col steps-j (descending); real time t0+i at
        #   col i+1; col steps+1 is the zero init.
        # Segments: [0..W], then 16-col segments covering the real cols.
        hbnd = [0, W + 1] + [W + 1 + 16 * i for i in range(1, ch // 16 + 1)] \
            + [steps + 2]
        hseg = {d: [big.tile([128, hbnd[i + 1] - hbnd[i], B], F16,
                             tag=f"h{d}{i}", name=f"h{d}{i}")
                    for i in range(len(hbnd) - 1)] for d in "fb"}

        def hcol(d, col):
            for i in range(len(hbnd) - 1):
                if col < hbnd[i + 1]:
                    return hseg[d][i][:, col - hbnd[i], :]
            raise AssertionError(col)

        def hspan2(d, c0):
            """[128, 2, B] span over cols c0, c0+1 (same segment)."""
            for i in range(len(hbnd) - 1):
                if c0 < hbnd[i + 1]:
                    assert c0 + 2 <= hbnd[i + 1], (d, c0)
                    return hseg[d][i][:, c0 - hbnd[i]:c0 - hbnd[i] + 2, :]
            raise AssertionError(c0)

        # ---- load inputs: idx first (feeds the gathers); weight/const
        # DMAs follow.
        nc.sync.dma_start(idx_sb[:], idx_d[:])
        nc.vector.memset(hcol("f", 0), 0.0)
        nc.vector.memset(hcol("b", steps + 1), 0.0)

        def load_consts():
            # ident first: it feeds the PE warm-up matmuls below
            nc.sync.dma_start(ident[:], ident_d[:])
            for d in "fb":
                nc.sync.dma_start(wihT[d][:], wihT_d[d].rearrange("(k p) g -> p k g", p=128))
                nc.sync.dma_start(whhT[d][:], whhT_d[d][:])
                nc.sync.dma_start(bias[d][:], bias_d[d][:])
                nc.sync.dma_start(biasw[d][:], biasw_d[d][:])
            nc.sync.dma_start(woutT[:], woutT_d.rearrange("c h t -> h c t"))
            nc.sync.dma_start(trans[:], trans_d.rearrange("p (i j) -> p i j", i=TAGS))

        # ---- embedding gathers, strictly in scan first-use order ----
        # Pipe-model completion times (us) for wait-pinning: the scheduler's
        # cost model underestimates SWDGE transfers ~2.5x, so gather-dependent
        # work would otherwise be placed too early in the in-order engine
        # streams and stall the scan on real HW.  lib load ~17.5us, then
        # 1.75us/quarter, 2.75us/half, 4.76us/full through the single queue.
        _GDUR = {"q": 1.75, "h": 2.75, "F": 4.76}
        gather_end = {}
        t_pipe = 17.5
        for g in _GATHERS:
            t_pipe += _GDUR[g[0]]
            gather_end[g] = t_pipe

        qslot = {}   # (block, q) -> xq slot
        hslot = {}   # (block, h) -> xh slot
        for g in _GATHERS:
            kind, b_, sub = g
            if kind == "q":
                qi = len(qslot)
                qslot[(b_, sub)] = qi
                nc.gpsimd.dma_gather(
                    xq[:, qi, :, :], emb_d[:, :],
                    idx_sb[:, 32 * b_ + 8 * sub:32 * b_ + 8 * (sub + 1)],
                    128, 128, EMB, transpose=True)
            elif kind == "h":
                hi = len(hslot)
                hslot[(b_, sub)] = hi
                nc.gpsimd.dma_gather(
                    xh[:, hi, :, :], emb_d[:, :],
                    idx_sb[:, 32 * b_ + 16 * sub:32 * b_ + 16 * (sub + 1)],
                    256, 256, EMB, transpose=True)
            else:
                nc.gpsimd.dma_gather(
                    xT[:, b_, :, :], emb_d[:, :],
                    idx_sb[:, 32 * b_:32 * (b_ + 1)],
                    512, 512, EMB, transpose=True)
            if g == _GATHERS[1]:
                # first two (step-0) pieces queued; now queue the SP-side
                # const DMAs so they land while the gathers run.
                load_consts()

        # ---- input projections: zin = x @ Wih.T + b ----
        zpsum = ctx.enter_context(tc.tile_pool(name="zpsum", bufs=2, space="PSUM"))
        _flip = [0]

        def _unit(d, kind, b_, sub):
            """One (dir, piece, gate-slice c x4) GEMM + bias copyback,
            pinned to the pipe-model time its gather really completes."""
            bsel = biasw[d] if _warm(d, kind, b_, sub) else bias[d]
            if kind == "q":
                src = xq[:, qslot[(b_, sub)], :, :]
                ncols, t0, nt = 128, TPB * b_ + 2 * sub, 2
            elif kind == "h":
                src = xh[:, hslot[(b_, sub)], :, :]
                ncols, t0, nt = 256, TPB * b_ + 4 * sub, 4
            else:
                src = xT[:, b_, :, :]
                ncols, t0, nt = 512, TPB * b_, 8
            for c in range(4):
                zp = zpsum.tile([128, 512], F32, tag="zp")
                nc.tensor.matmul(
                    zp[:, :ncols], wihT[d][:, 0, 128 * c:128 * (c + 1)],
                    src[:, 0, :], start=True, stop=False)
                nc.tensor.matmul(
                    zp[:, :ncols], wihT[d][:, 1, 128 * c:128 * (c + 1)],
                    src[:, 1, :], start=False, stop=True)
                dst = zin[d][:, t0:t0 + nt, c, :]
                if _flip[0] % 2 == 0:
                    nc.scalar.activation(dst, zp[:, :ncols], AF.Identity,
                                         bias=bsel[:, c:c + 1])
                else:
                    nc.vector.tensor_scalar(dst, zp[:, :ncols], bsel[:, c:c + 1],
                                            None, ALU.add)
                _flip[0] += 1

        # PE HAM warm-up: the PE would otherwise idle through the ~17us
        # gpsimd ucode load and enter the first GEMMs / scan steps at the
        # cold 1.2 GHz clock.  Dummy ident@ident matmuls during that dead
        # window push it to 8/8 (2.4 GHz) before real work arrives.
        for g_ in range(5):
            wz = zpsum.tile([128, 128], F32, tag="zp", name="wz")
            for i in range(8):
                nc.tensor.matmul(wz[:], ident[:], ident[:],
                                 start=(i == 0), stop=(i == 7))

        # All units upfront, pinned to their gather's pipe-model completion.
        # (In-loop emission of far-future units was tried: it spreads the
        # work but lands it on the emission wave and mid-scan, net worse.)
        for d, kind, b_, sub, s_use in _UNITS:
            t_ready = gather_end[(kind, b_, sub)] + 0.3
            with tc.tile_wait_until(t_ready / 1000.0):
                _unit(d, kind, b_, sub)

        # ---- the recurrent scan (fwd + bwd interleaved) ----
        with tc.tile_pool(name="spsum", bufs=4, space="PSUM") as spsum, \
             tc.tile_pool(name="sQ", bufs=10) as sQ, \
             tc.tile_pool(name="sP", bufs=10) as sP, \
             tc.tile_pool(name="sT", bufs=8) as sT:

            def new_z(k):
                """Fresh psum tiles for step k with zin injected (identity mm).
                Emitted one step ahead so gate mms fire as soon as h lands."""
                zt = {}
                for d in "fb":
                    ti = k if d == "f" else nts - 1 - k
                    zt[d] = spsum.tile([128, 4, B], F32, tag="z", name=f"z{d}")
                    nc.tensor.matmul(zt[d][:], ident[:],
                                     zin[d][:, ti, :, :],
                                     start=True, stop=False)
                return zt

            q = {d: sQ.tile([128, 5, B], F32, tag="q", name=f"q{d}") for d in "fb"}
            for d in "fb":
                nc.vector.memset(q[d][:, 4, :], 0.0)
            z = new_z(0)
            for k in range(steps):
                for d, rd_col in (("f", k), ("b", steps + 1 - k)):
                    for c in range(4):
                        nc.tensor.matmul(
                            z[d][:, c, :],
                            whhT[d][:, 128 * c:128 * (c + 1)],
                            hcol(d, rd_col),
                            start=False, stop=(c == 3))
                z_cur, z = z, (new_z(k + 1) if k + 1 < steps else None)
                qn = {d: sQ.tile([128, 5, B], F32, tag="q", name=f"q{d}")
                      for d in "fb"}
                # Stage-interleaved emission: both dirs' ops alternate at
                # each chain stage so the engine queues lock the two chains
                # half a period out of phase.
                wr_col = {"f": k + 1, "b": steps - k}
                P = {}
                for d in "fb":
                    # T = tanh(z') into blocks [o,i,f,g]; state cc = 2*c
                    # sits in block 4.
                    nc.scalar.activation(q[d][:, 0:4, :], z_cur[d][:],
                                         AF.Tanh)
                for d in "fb":
                    # P = [(T_i+1)*T_g | (T_f+1)*cc]
                    P[d] = sP.tile([128, 2, B], F32, tag="P", name="P")
                    nc.vector.scalar_tensor_tensor(
                        P[d][:], q[d][:, 1:3, :], 1.0, q[d][:, 3:5, :],
                        ALU.add, ALU.mult)
                for d in "fb":
                    # cc_new = 2*c_new = 0.5*P1 + P0, written straight into
                    # the next step's Q tile (no separate state-fix op).
                    nc.vector.scalar_tensor_tensor(
                        qn[d][:, 4, :], P[d][:, 1, :], 0.5, P[d][:, 0, :],
                        ALU.mult, ALU.add)
                TC = {}
                for d in "fb":
                    TC[d] = sT.tile([128, B], F32, tag="TC", name="TC")
                    nc.scalar.activation(TC[d][:], qn[d][:, 4, :], AF.Tanh,
                                         scale=0.5)
                for d in "fb":
                    # h' = (T_o + 1) * TC
                    nc.vector.scalar_tensor_tensor(
                        hcol(d, wr_col[d]), q[d][:, 0, :], 1.0, TC[d][:],
                        ALU.add, ALU.mult)
                q = qn

        # ---- emission + CRF broadcast-add + store ----
        # chunk n covers local times 2n, 2n+1 (128 tokens);
        # hf cols W+1+2n..W+2+2n, hb cols 2n+1..2n+2.
        # (256-token chunks were tried: the bigger DVE op concentrates the
        # tail intrusions and lengthens the post-scan chain — net worse.)
        nchunks = ch // 2
        order = sorted(range(nchunks),
                       key=lambda n: max(W + 2 + 2 * n, steps - 1 - 2 * n))
        for n in order:
            e = epsum.tile([128, TAGS], F32, tag="e")
            nc.tensor.matmul(e[:], hspan2("f", W + 1 + 2 * n),
                             woutT[:, 0, :], start=True, stop=False)
            nc.tensor.matmul(e[:], hspan2("b", 1 + 2 * n),
                             woutT[:, 1, :], start=False, stop=True)
            crf_sb = ecrf.tile([128, TAGS, TAGS], F32, tag="crf")
            e_b = e[:, None, :].to_broadcast([128, TAGS, TAGS])
            nc.vector.tensor_tensor(crf_sb[:], e_b, trans[:], ALU.add)
            nc.sync.dma_start(crf_d[128 * n:128 * (n + 1), :], crf_sb[:])

    nc.compile()
    _assert_ldw_pairing(nc)
    return nc


def _assert_ldw_pairing(nc):
    """Every non-self-loading matmul must directly follow an InstLdweights
    whose weights AP matches the matmul's weights operand."""
    for f in nc.m.functions:
        for bb in f.blocks:
            prev_pe = None
            for ins in bb.instructions:
                if ins.engine != mybir.EngineType.PE:
                    continue
                if isinstance(ins, mybir.InstMatmult) and ins.ldweights is False:
                    assert isinstance(prev_pe, mybir.InstLdweights), (
                        f"{ins.name}: non-self-loading matmul not preceded by "
                        f"ldweights (got {type(prev_pe).__name__})")
                    assert repr(prev_pe.ins[0]) == repr(ins.ins[1]), (
                        f"{ins.name}: weights mismatch with {prev_pe.name}")
                prev_pe = ins


_CACHE = {}


def _get_nc():
    if "nc" not in _CACHE:
        _CACHE["nc"] = build()
    return _CACHE["nc"]


def _prep_dir(w_ih, w_hh, b):
    """Permute gates to [o,i,f,g]; apply tanh-half trick (f,i,o rows x0.5)
    and h'=2h compensation (all Whh x0.5)."""
    w_ih = np.asarray(w_ih, np.float32)[_PERM] * _BLK_SCALE[:, None]
    w_hh = np.asarray(w_hh, np.float32)[_PERM] * (0.5 * _BLK_SCALE[:, None])
    b = np.asarray(b, np.float32)[_PERM] * _BLK_SCALE
    wihT = np.ascontiguousarray(w_ih.T).astype(np.float16)
    whhT = np.ascontiguousarray(w_hh.T).astype(np.float16)
    bias = np.ascontiguousarray(b.reshape(4, 128).T).astype(np.float32)
    return wihT, whhT, bias


def make_in_maps(sentences, embedding, W_ih_f, W_hh_f, b_f, W_ih_b, W_hh_b,
                 b_b, W_out, b_out, transition):
    emb = np.zeros((VPAD, EMB), np.float32)
    emb[:VOCAB] = np.asarray(embedding, np.float32)
    emb = emb.astype(np.float16)
    wihT_f, whhT_f, bias_f = _prep_dir(W_ih_f, W_hh_f, b_f)
    wihT_b, whhT_b, bias_b = _prep_dir(W_ih_b, W_hh_b, b_b)
    wo = np.asarray(W_out, np.float32) * 0.5   # h' = 2h compensation
    woutT = np.stack([np.ascontiguousarray(wo[:, :128].T),
                      np.ascontiguousarray(wo[:, 128:].T)])
    woutT = woutT.astype(np.float16)  # [2, 128, 16]
    trans_aug = (np.asarray(transition, np.float32)
                 + np.asarray(b_out, np.float32)[None, :]).reshape(-1)  # [256]
    trans_rep = np.ascontiguousarray(
        np.broadcast_to(trans_aug, (128, 256))).astype(np.float32)
    ident = np.eye(128, dtype=np.float16)
    zeros4 = np.zeros((128, 4), np.float32)

    # tokens per core: times [64c - W, 64c + 64 + W), batch-inner (t, b)
    # order; out-of-range times -> the zero embedding row (ZTOK).
    sent = np.asarray(sentences).astype(np.int64)  # [B, T]
    in_maps = []
    for c in range(NCORES):
        t_lo = CH * c - W
        times = np.arange(t_lo, t_lo + NTS)
        cols = np.clip(times, 0, T - 1)
        toks = sent[:, cols].T.copy()          # [NTS, B]
        toks[(times < 0) | (times >= T)] = ZTOK
        toks = toks.reshape(-1)                # (t, b) order, [NTOK]
        idx = np.tile(toks.reshape(NTOK // 16, 16).T.astype(np.int16), (8, 1))
        in_maps.append({
            "emb": emb, "idx": idx,
            "wihT_f": wihT_f, "wihT_b": wihT_b,
            "whhT_f": whhT_f, "whhT_b": whhT_b,
            "bias_f": bias_f, "bias_b": bias_b,
            "biasw_f": zeros4 if c == 0 else bias_f,
            "biasw_b": zeros4 if c == NCORES - 1 else bias_b,
            "woutT": woutT, "trans": trans_rep, "ident": ident,
        })
    return in_maps


def assemble_out(results):
    out = np.empty((B, T, TAGS, TAGS), np.float32)
    for c in range(NCORES):
        crf = results[c]["crf"].reshape(CH, B, TAGS, TAGS)
        out[:, CH * c:CH * (c + 1)] = crf.transpose(1, 0, 2, 3)
    return out


def kernel(**inputs):
    _ensure_ntff_hook()
    nc = _get_nc()
    in_maps = make_in_maps(**inputs)
    res = run_bass_kernel_spmd(nc, in_maps, list(range(NCORES)))
    return assemble_out(res.results)
